# revision 37
# baseline (speedup 1.0000x reference)
"""BiMamba block kernel for 8 Trainium2 NeuronCores.

Sharding: core = 2*sample + direction (4 samples x 2 scan directions).
Each core runs the full mamba for its (sample, direction): input/gate
projections (PE), causal depthwise conv (PE diag-matmul), dt softplus
(ACT), then 16 selective-scan states via the DVE tensor_tensor_scan
instruction, with exp(dt*A) on ACT, the dt*x*B / C*h products on DVE in
bf16, and the sum over states accumulated in PSUM by identity matmuls.
The sequence is processed in two halves so the tail of half 0 (gating,
out-projection, partial 3x3 conv) overlaps the scan of half 1.  The
block tail (3x3 conv partials, BatchNorm batch stats, residual,
LeakyReLU) uses a pair AllReduce plus an 8-core stats AllReduce.
"""
import os
import sys

for _p in ("/opt/trn_rl_repo", "/root/.axon_site/_ro/trn_rl_repo"):
    if os.path.isdir(_p):
        if _p not in sys.path:
            sys.path.insert(0, _p)
        break

import ml_dtypes
import numpy as np

# The agent image's antenv lacks axon_hooks; inject it so trace=True can
# capture NTFF profiles (used by test.py for HW timing, not for grading).
try:
    import antenv.axon_hooks  # noqa: F401
except ImportError:
    try:
        import types as _types

        from trn_agent_boot.trn_boot import _ntff_profile_via_ctypes

        _hook = _ntff_profile_via_ctypes("/opt/axon/libaxon_pjrt.so")
        _m = _types.ModuleType("antenv.axon_hooks")
        _m.get_axon_ntff_profile_hook = lambda: _hook
        _m.set_axon_ntff_profile_hook = lambda h: None
        sys.modules["antenv.axon_hooks"] = _m
    except Exception:
        pass

import concourse.bass as bass
import concourse.mybir as mybir
from concourse import bacc
from concourse import bass_utils
from concourse.masks import make_identity
from concourse.tile import TileContext

F32 = mybir.dt.float32
BF16 = mybir.dt.bfloat16
AF = mybir.ActivationFunctionType
OP = mybir.AluOpType

B, C, H, W = 4, 64, 64, 64
L = H * W          # 4096
DI = 128           # d_inner
DS = 16            # d_state
DTR = 4            # dt_rank
DCONV = 4
NCORE = 8
CH = 512           # matmul free-dim chunk
NCH = L // CH      # 8
HALF = L // 2      # scan chunk length
NHALF = 2
RPC = CH // W      # output rows per chunk (8)


def _build():
    nc = bacc.Bacc(target_bir_lowering=False, debug=False, num_devices=NCORE)

    def din(name, shape, dtype=F32):
        return nc.dram_tensor(name, shape, dtype, kind="ExternalInput")

    F32R = mybir.dt.float32r
    x_loc = din("x_loc", [C, L], F32R)
    # all fp32/f32r params packed into one blob (single DMA), bf16 in another
    blob_f = din("blob_f", [128, 508], F32R)
    blob_h = din("blob_h", [128, 9 * C + C], BF16)

    out_d = nc.dram_tensor("out", [C, L], F32, kind="ExternalOutput")

    with TileContext(nc) as tc:
        with tc.tile_pool(name="pers", bufs=1) as pers:
            # ---- params arrive as two packed blobs ----
            p_bf = pers.tile([128, 508], F32R)
            p_bh = pers.tile([128, 9 * C + C], BF16)
            nc.sync.dma_start(p_bf[:], blob_f[:])
            nc.sync.dma_start(p_bh[:], blob_h[:])
            p_in_wT = p_bf[:, 0:256]
            p_c1w = p_bf[:, 256:260].bitcast(F32)
            p_c1b = p_bf[:, 260:261].bitcast(F32)
            p_bigT = p_bf[:, 261:389]
            p_bcwT = p_bf[:, 389:421]
            p_dtb = p_bf[:, 421:422].bitcast(F32)
            p_A = p_bf[:, 422:438].bitcast(F32)
            p_D = p_bf[:, 438:439].bitcast(F32)
            p_rwT = p_bf[:, 439:503]
            p_c3b = p_bf[:C, 504:505].bitcast(F32)
            p_rb = p_bf[:C, 505:506].bitcast(F32)
            p_bng = p_bf[:C, 506:507].bitcast(F32)
            p_bnb = p_bf[:C, 507:508].bitcast(F32)
            p_c3w = p_bh[:, 0:9 * C]
            p_owT = p_bh[:, 9 * C:9 * C + C]

            ident = pers.tile([128, 128], F32)
            make_identity(nc, ident[:])
            ident_g = pers.tile([128, 128], BF16)
            nc.vector.tensor_copy(ident_g[:], ident[:])
            diag_c1 = [pers.tile([128, 128], F32R, tag=f"dgc{k}", name=f"dgc{k}")
                       for k in range(DCONV)]
            for k in range(DCONV):
                nc.vector.tensor_scalar_mul(diag_c1[k][:], ident[:],
                                            p_c1w[:, k:k + 1])

            # DRAM staging for B/C rows (DMA partition-broadcast needs a
            # DRAM source)
            bc_dram = nc.dram_tensor("bc_stage", [2 * DS, L], BF16)
            y_gated = pers.tile([DI, L], BF16)

            with tc.tile_pool(name="smid", bufs=1) as smid:
                z_sil = smid.tile([DI, L], BF16)
                dtv = smid.tile([DI, L], F32)
                dtxc_bf = smid.tile([DI, L], BF16)
                xcd = smid.tile([DI, L], BF16)
                carry = smid.tile([DI, DS], F32)
                res_sb = smid.tile([C, L], F32)

                with tc.tile_pool(name="sl_a", bufs=3) as pla, \
                     tc.tile_pool(name="sl_b", bufs=3) as plb, \
                     tc.tile_pool(name="sl_x", bufs=3) as plx, \
                     tc.tile_pool(name="sl_h", bufs=2) as plh, \
                     tc.tile_pool(name="sl_c", bufs=3) as plc, \
                     tc.tile_pool(name="sl_g", bufs=2) as plg:
                  with tc.tile_pool(name="ph12", bufs=1) as p12, \
                       tc.tile_pool(name="psA", bufs=3, space="PSUM") as psA:
                      x_sb = p12.tile([128, L], F32R)
                      nc.gpsimd.memset(x_sb[64:128, :].bitcast(F32), 0.0)
                      nc.sync.dma_start(x_sb[0:64, :], x_loc[:])
                      xi_pad = p12.tile([DI, 3 + L], F32R)
                      nc.gpsimd.memset(xi_pad[:, 0:3].bitcast(F32), 0.0)
                      xc = p12.tile([DI, L], F32R)

                      # front-end phases, emitted per sequence-half so the
                      # second half's projections overlap the first half's scan
                      xc_f = xc[:].bitcast(F32)
                      for hf2 in range(NHALF):
                          cr = range(hf2 * (NCH // 2), (hf2 + 1) * (NCH // 2))
                          # phase 1: xz projection + silu(z)
                          for c in cr:
                              sl = slice(c * CH, (c + 1) * CH)
                              ps = psA.tile([128, CH], F32, tag="ps")
                              nc.tensor.matmul(ps[:DI], p_in_wT[:, 0:DI],
                                               x_sb[:, sl], start=True,
                                               stop=True)
                              nc.scalar.copy(
                                  xi_pad[:, 3 + c * CH:3 + (c + 1) * CH],
                                  ps[:DI])
                              ps2 = psA.tile([128, CH], F32, tag="ps")
                              nc.tensor.matmul(ps2[:DI], p_in_wT[:, DI:2 * DI],
                                               x_sb[:, sl], start=True,
                                               stop=True)
                              nc.scalar.activation(z_sil[:, sl], ps2[:DI],
                                                   AF.Silu)
                          # phase 2: causal depthwise conv1d + silu
                          for c in cr:
                              sl = slice(c * CH, (c + 1) * CH)
                              ps = psA.tile([128, CH], F32, tag="ps")
                              for k in range(DCONV):
                                  nc.tensor.matmul(
                                      ps[:DI], diag_c1[k][:],
                                      xi_pad[:, c * CH + k:c * CH + k + CH],
                                      start=(k == 0), stop=(k == DCONV - 1))
                              nc.scalar.activation(xc[:, sl], ps[:DI], AF.Silu,
                                                   bias=p_c1b[:, 0:1])
                          # phase 3: dt pre-activation + B/C projection
                          for c in cr:
                              sl = slice(c * CH, (c + 1) * CH)
                              ps = psA.tile([128, CH], F32, tag="ps")
                              nc.tensor.matmul(ps[:DI], p_bigT[:], xc[:, sl],
                                               start=True, stop=True)
                              # softplus = ln(1 + exp(.))
                              nc.scalar.activation(dtv[:, sl], ps[:DI], AF.Exp,
                                                   bias=p_dtb[:, 0:1])
                              ps2 = psA.tile([128, CH], F32, tag="ps")
                              nc.tensor.matmul(ps2[:2 * DS], p_bcwT[:],
                                               xc[:, sl], start=True, stop=True)
                              bch = plb.tile([2 * DS, CH], BF16, tag="bch")
                              nc.vector.tensor_copy(bch[:], ps2[:2 * DS])
                              nc.sync.dma_start(bc_dram[:, sl], bch[:])
                          hsl2 = slice(hf2 * HALF, (hf2 + 1) * HALF)
                          nc.scalar.activation(dtv[:, hsl2], dtv[:, hsl2],
                                               AF.Ln, bias=1.0)
                          # phase 4 per half: dt*xc (bf16) and xc*D (bf16)
                          nc.vector.tensor_mul(dtxc_bf[:, hsl2], dtv[:, hsl2],
                                               xc_f[:, hsl2])
                          nc.scalar.activation(xcd[:, hsl2], xc_f[:, hsl2],
                                               AF.Copy, scale=p_D[:, 0:1])
                      # residual projection (needs only x_sb; PE fills gaps
                      # behind the scan phase)
                      for c in range(NCH):
                          slr = slice(c * CH, (c + 1) * CH)
                          psr = psA.tile([128, CH], F32, tag="ps")
                          nc.tensor.matmul(psr[:C], p_rwT[:], x_sb[:, slr],
                                           start=True, stop=True)
                          nc.scalar.activation(res_sb[:, slr], psr[:C],
                                               AF.Identity, bias=p_rb[:, 0:1])

                  # ---- selective scan + overlapped tail ----
                  with tc.tile_pool(name="pp", bufs=8, space="PSUM") as pp, \
                       tc.tile_pool(name="tail", bufs=1) as tl, \
                       tc.tile_pool(name="dram", bufs=1, space="DRAM") as dr:
                      ympad = tl.tile([128, H + 2, W + 2], BF16)
                      nc.gpsimd.memset(ympad[:], 0.0)
                      conv_part = tl.tile([C, L], BF16)
                      PAIRS = [[0, 1], [2, 3], [4, 5], [6, 7]]

                      def conv3_chunk(c):
                          ps = pp.tile([128, CH], F32, tag="pp",
                                       name=f"cv{c}")
                          ps3 = ps[:C].rearrange("p (r w) -> p r w", w=W)
                          r0 = c * RPC
                          n = 0
                          for ky in range(3):
                              for kx in range(3):
                                  nc.tensor.matmul(
                                      ps3[:],
                                      p_c3w[:, (ky * 3 + kx) * C:
                                            (ky * 3 + kx + 1) * C],
                                      ympad[:, r0 + ky:r0 + ky + RPC, kx:kx + W],
                                      start=(n == 0), stop=(n == 8))
                                  n += 1
                          nc.scalar.activation(conv_part[:, c * CH:(c + 1) * CH],
                                               ps3.rearrange("p r w -> p (r w)"),
                                               AF.Identity, bias=p_c3b[:, 0:1])

                      SEGS = ((0, HALF, (0, 1, 2)), (HALF, L, (3, 4, 5, 6, 7)))
                      cc_ins, cc_outs = [], []
                      for gi, (t0, t1, cvs) in enumerate(SEGS):
                          cc_ins.append(dr.tile([C, len(cvs) * CH], BF16, name=f"cci{gi}"))
                          cc_outs.append(dr.tile([C, len(cvs) * CH], BF16, name=f"cco{gi}"))
                      conv_full = tl.tile([C, L], BF16)
                      stats2 = tl.tile([C, 3, 2], F32)
                      st_in = dr.tile([C, 2], F32)
                      st_out = nc.dram_tensor("st_out", [C, 2], F32,
                                              addr_space="Shared")

                      for gi, (t0, t1, cvs) in enumerate(SEGS):
                          seg = t1 - t0
                          chunks = range(t0 // CH, t1 // CH)
                          y_ps = {}
                          for cix in chunks:
                              yp = pp.tile([128, CH], F32, tag="pp", name=f"y{cix}")
                              nc.tensor.matmul(yp[:DI], ident_g[:],
                                               xcd[:, cix * CH:(cix + 1) * CH],
                                               start=True, stop=False)
                              y_ps[cix] = yp

                          hsl = slice(t0, t1)
                          for s in range(DS):
                              da = pla.tile([DI, seg], F32, tag="da")
                              nc.scalar.activation(da[:], dtv[:, hsl], AF.Exp,
                                                   scale=p_A[:, s:s + 1])
                              bbc = plb.tile([DI, seg], BF16, tag="bbc")
                              if gi == 0 and s < 2:
                                  # chunk-granular broadcast for the first
                                  # states: each piece fires as soon as its
                                  # chunk's bc staging lands, so the first
                                  # scan isn't gated on the whole half
                                  for cj in range(t0 // CH, t1 // CH):
                                      cs = slice(cj * CH, (cj + 1) * CH)
                                      nc.sync.dma_start(
                                          bbc[:, cj * CH - t0:
                                              (cj + 1) * CH - t0],
                                          bc_dram[s:s + 1, cs].to_broadcast(
                                              (DI, CH)))
                              else:
                                  nc.sync.dma_start(
                                      bbc[:],
                                      bc_dram[s:s + 1, hsl].to_broadcast(
                                          (DI, seg)))
                              dbx = plx.tile([DI, seg], BF16, tag="dbx")
                              nc.vector.tensor_mul(dbx[:], dtxc_bf[:, hsl], bbc[:])
                              h = plh.tile([DI, seg], BF16, tag="h")
                              init = 0.0 if gi == 0 else carry[:, s:s + 1]
                              nc.vector.tensor_tensor_scan(h[:], da[:], dbx[:], init,
                                                           op0=OP.mult, op1=OP.add)
                              if gi < len(SEGS) - 1:
                                  nc.scalar.copy(carry[:, s:s + 1], h[:, seg - 1:seg])
                              cbc = plc.tile([DI, seg], BF16, tag="cbc")
                              nc.sync.dma_start(
                                  cbc[:],
                                  bc_dram[DS + s:DS + s + 1, hsl].to_broadcast((DI, seg)))
                              g = plg.tile([DI, seg], BF16, tag="g")
                              nc.vector.tensor_mul(g[:], h[:], cbc[:])
                              for j, cix in enumerate(chunks):
                                  nc.tensor.matmul(y_ps[cix][:DI], ident_g[:],
                                                   g[:, j * CH:(j + 1) * CH],
                                                   start=False, stop=(s == DS - 1))

                          # gating + out-projection + padded spatial write
                          for cix in chunks:
                              sl = slice(cix * CH, (cix + 1) * CH)
                              nc.vector.tensor_mul(y_gated[:, sl], y_ps[cix][:DI],
                                                   z_sil[:, sl])
                              po = pp.tile([128, CH], F32, tag="pp", name=f"po{cix}")
                              nc.tensor.matmul(po[:C], p_owT[:], y_gated[:, sl],
                                               start=True, stop=True)
                              r0 = cix * RPC
                              nc.scalar.copy(ympad[0:C, 1 + r0:1 + r0 + RPC, 1:1 + W],
                                             po[:C].rearrange("p (r w) -> p r w", w=W))

                          # 3x3 conv on rows whose inputs are now complete, pair
                          # AllReduce that span, and its BN partial sums
                          for c in cvs:
                              conv3_chunk(c)
                          lo, hi = cvs[0] * CH, (cvs[-1] + 1) * CH
                          nc.sync.dma_start(cc_ins[gi][:], conv_part[:, lo:hi])
                          nc.gpsimd.collective_compute(
                              "AllReduce", OP.add, replica_groups=PAIRS,
                              ins=[cc_ins[gi][:].opt()], outs=[cc_outs[gi][:].opt()])
                          nc.sync.dma_start(conv_full[:, lo:hi], cc_outs[gi][:])
                          nc.vector.tensor_reduce(stats2[:, gi, 0:1], conv_full[:, lo:hi],
                                                  axis=mybir.AxisListType.X, op=OP.add)
                          # square scratch overwrites conv_part (dead after the
                          # collective input DMA); only accum_out is consumed
                          nc.scalar.activation(conv_part[:, lo:hi], conv_full[:, lo:hi],
                                               AF.Square, accum_out=stats2[:, gi, 1:2])

                      stats = tl.tile([C, 2], F32)
                      nc.vector.tensor_add(stats[:], stats2[:, 0],
                                           stats2[:, 1])
                      for gi in range(2, len(SEGS)):
                          nc.vector.tensor_add(stats[:], stats[:],
                                               stats2[:, gi])
                      nc.sync.dma_start(st_in[:], stats[:])
                      nc.gpsimd.collective_compute(
                          "AllReduce", OP.add,
                          replica_groups=[[0, 1, 2, 3, 4, 5, 6, 7]],
                          ins=[st_in[:].opt()], outs=[st_out[:].opt()])
                      stot = tl.tile([C, 2], F32)
                      nc.sync.dma_start(stot[:], st_out[:])

                      inv_n = 1.0 / (2.0 * B * L)
                      mean = tl.tile([C, 1], F32)
                      var = tl.tile([C, 1], F32)
                      tmp = tl.tile([C, 1], F32)
                      nc.vector.tensor_scalar_mul(mean[:], stot[:, 0:1], inv_n)
                      nc.vector.tensor_scalar_mul(var[:], stot[:, 1:2], inv_n)
                      nc.vector.tensor_mul(tmp[:], mean[:], mean[:])
                      nc.vector.tensor_sub(var[:], var[:], tmp[:])
                      # invstd = 1/sqrt(var + eps)
                      nc.vector.tensor_scalar_add(var[:], var[:], 1e-5)
                      nc.scalar.activation(tmp[:], var[:], AF.Sqrt)
                      nc.vector.reciprocal(tmp[:], tmp[:])
                      scal = tl.tile([C, 1], F32)
                      shft = tl.tile([C, 1], F32)
                      nc.vector.tensor_mul(scal[:], p_bng[:], tmp[:])
                      nc.vector.tensor_mul(tmp[:], mean[:], scal[:])
                      nc.vector.tensor_sub(shft[:], p_bnb[:], tmp[:])

                      # bn + residual + leaky relu:
                      #   out = prelu(conv*scal + res + shft)
                      # 8 blocks pipeline DVE stt -> ACT prelu -> out DMA so
                      # the final DMA trails only one 512-col block
                      bn1 = tl.tile([128, L], F32, tag="xbn", name="bn1")[:C]
                      for c8 in range(NCH):
                          lo, hi = c8 * CH, (c8 + 1) * CH
                          nc.vector.scalar_tensor_tensor(
                              bn1[:, lo:hi], conv_full[:, lo:hi],
                              scal[:, 0:1], res_sb[:, lo:hi],
                              op0=OP.mult, op1=OP.add)
                          nc.scalar.activation(bn1[:, lo:hi], bn1[:, lo:hi],
                                               AF.Prelu, alpha=0.01,
                                               bias=shft[:, 0:1])
                          nc.sync.dma_start(out_d[:, lo:hi], bn1[:, lo:hi])

    nc.compile()
    return nc


_NC = None


def _get_nc():
    global _NC
    if _NC is None:
        _NC = _build()
    return _NC


def _prep_in_maps(inp):
    inp = {k: np.asarray(v, dtype=np.float32) for k, v in inp.items()}
    x = inp["x"]  # (4, 64, 64, 64)
    maps = []
    for core in range(NCORE):
        b, d = core // 2, core % 2
        pre = "m1_" if d == 0 else "m2_"
        in_w = inp[pre + "in_w"]          # (256, 64)
        xproj_w = inp[pre + "xproj_w"]    # (36, 128)
        dt_w = inp[pre + "dt_w"]          # (128, 4)

        x_loc = x[b].reshape(C, L)
        if d == 1:
            x_loc = x_loc[:, ::-1]

        in_wT = np.zeros((128, 2 * DI), np.float32)
        in_wT[:C] = in_w.T
        bigproj = dt_w @ xproj_w[:DTR]    # (128, 128)
        conv3_slice = inp["conv_w"][:, d * C:(d + 1) * C]  # (64,64,3,3)
        c3 = np.zeros((128, 9 * C), np.float32)
        for ky in range(3):
            for kx in range(3):
                c3[:C, (ky * 3 + kx) * C:(ky * 3 + kx + 1) * C] = \
                    conv3_slice[:, :, ky, kx].T
        res_wT = np.zeros((128, C), np.float32)
        res_wT[:C] = inp["res_w"].T

        blob_f = np.zeros((128, 508), np.float32)
        blob_f[:, 0:256] = in_wT
        blob_f[:, 256:260] = inp[pre + "conv_w"]
        blob_f[:, 260] = inp[pre + "conv_b"]
        blob_f[:, 261:389] = bigproj.T
        blob_f[:, 389:421] = xproj_w[DTR:].T
        blob_f[:, 421] = inp[pre + "dt_b"]
        blob_f[:, 422:438] = -np.exp(inp[pre + "A_log"])
        blob_f[:, 438] = inp[pre + "D"]
        blob_f[:, 439:503] = res_wT
        blob_f[:C, 504] = (inp["conv_b"] if d == 0
                           else np.zeros_like(inp["conv_b"]))
        blob_f[:C, 505] = inp["res_b"]
        blob_f[:C, 506] = inp["bn_gamma"]
        blob_f[:C, 507] = inp["bn_beta"]
        blob_h = np.zeros((128, 9 * C + C), np.float32)
        blob_h[:, 0:9 * C] = c3
        blob_h[:, 9 * C:] = inp[pre + "out_w"].T
        m = {
            "x_loc": np.ascontiguousarray(x_loc),
            "blob_f": blob_f,
            "blob_h": blob_h.astype(ml_dtypes.bfloat16),
        }
        maps.append(m)
    return maps


def _run(inputs, trace=False):
    nc = _get_nc()
    maps = _prep_in_maps(inputs)
    res = bass_utils.run_bass_kernel_spmd(
        nc, maps, core_ids=list(range(NCORE)), trace=trace)
    out = np.stack([res.results[2 * b]["out"].reshape(C, H, W)
                    for b in range(B)])
    return out, res


def kernel(**inputs) -> np.ndarray:
    out, _ = _run(inputs, trace=False)
    return out

